# revision 3
# baseline (speedup 1.0000x reference)
# Trainium2 raw-Bass kernel for nn_GraphStack (gnn_message_passing).
#
# Math per layer (B=8, N=2048, F=128, L=2):
#   w1 = lrelu(x @ W3); w2 = lrelu(x @ W4)
#   wt = w1 @ w2^T ; msg = (wt @ x - diag(w1.w2)*x)/(N-1)
#   x = lrelu(msg @ W5) + x
#
# Restructure: wt @ x = w1 @ (w2^T @ x) — the N x N pairwise matrix is
# never formed; S = w2^T x is 128x128. diag term handled exactly via
# z = -diag*x accumulated into msg^T with PE transpose-matmuls.
# 1/(N-1) folded into W5 host-side. Biases are zeros by spec fill.
#
# Sharding: data-parallel, batch element b -> core b, no collectives.
#
# Raw Bass (not Tile): this container's walrus rejects instructions with
# more than one attached sync-wait, which Tile's scheduler and epilogue
# drain always produce. Here every cross-engine dependency is a
# standalone wait_ge on one of four monotonic semaphores.

import numpy as np
from contextlib import ExitStack

import concourse.bass as bass
import concourse.mybir as mybir
from concourse.bass_utils import run_bass_kernel_spmd

B, N, F, L = 8, 2048, 128, 2
NCH = N // 128
SLOPE = 0.1
FP = mybir.dt.float32
AF = mybir.ActivationFunctionType
ALU = mybir.AluOpType
ts = bass.ts

_CACHE = {}


def _build_nc(reps=1):
    RL = reps * L  # total layers emitted; weights cycle mod L
    nc = bass.Bass()

    x_d = nc.declare_dram_parameter("x", [N, F], FP, isOutput=False)
    xT_d = nc.declare_dram_parameter("xT", [F, N], FP, isOutput=False)
    w34_d = nc.declare_dram_parameter("w34", [L, F, 2 * F], FP, isOutput=False)
    w5_d = nc.declare_dram_parameter("w5", [L, F, F], FP, isOutput=False)
    id_d = nc.declare_dram_parameter("ident", [F, F], FP, isOutput=False)
    y_d = nc.declare_dram_parameter("y", [N, F], FP, isOutput=True)

    ctx = ExitStack()
    sb = lambda shape, name: ctx.enter_context(nc.sbuf_tensor(name, shape, FP))
    w34_sb = sb([F, L * 2 * F], "w34_sb")
    w5_sb = sb([F, L * F], "w5_sb")
    ident = sb([F, F], "ident_sb")
    xnat = [sb([128, N], f"xnat{i}") for i in range(2)]
    xT = [sb([F, N], f"xT{i}") for i in range(2)]
    w1n = sb([128, N], "w1n")
    w2n = sb([128, N], "w2n")
    prodneg = sb([128, N], "prodneg")
    z = sb([128, N], "z")
    w1T = sb([F, N], "w1T")
    msgT = sb([F, N], "msgT")
    negdiag = sb([128, NCH], "negdiag")
    S_sb = sb([F, F], "S_sb")
    psall = ctx.enter_context(nc.psum_tensor("psall", [128, 2 * N], FP))
    ps0 = psall[:, 0:N]
    ps1 = psall[:, N : 2 * N]

    s_dma = ctx.enter_context(nc.semaphore("s_dma"))
    s_pe = ctx.enter_context(nc.semaphore("s_pe"))
    s_act = ctx.enter_context(nc.semaphore("s_act"))
    s_dve = ctx.enter_context(nc.semaphore("s_dve"))

    # ---- milestone schedule (python-side counters) ----
    # dma: each dma_start +16
    DMA_LOADS = 80  # w3,w4,w5,ident,x,xT
    DMA_X = 80  # first five loads incl. x natural
    # pe / act / dve milestones per layer, computed below
    pe_c, act_c, dve_c = [0], [0], [0]

    def nxt(c):
        c[0] += 1
        return c[0]

    # precompute milestone numbers in emission order
    M = {}
    for l in range(RL):
        last = l == RL - 1
        M[f"pe_AB{l}"] = nxt(pe_c)
        M[f"pe_C{l}"] = nxt(pe_c)
        M[f"pe_S{l}"] = nxt(pe_c)
        M[f"pe_D{l}"] = nxt(pe_c)
        if not last:
            M[f"pe_E{l}"] = nxt(pe_c)
        M[f"pe_F{l}"] = nxt(pe_c)
        M[f"a_w1n{l}"] = nxt(act_c)
        M[f"a_w2n{l}"] = nxt(act_c)
        M[f"a_w1T{l}"] = nxt(act_c)
        M[f"a_msgT{l}"] = nxt(act_c)
        if not last:
            M[f"a_E{l}"] = nxt(act_c)
        M[f"a_F{l}"] = nxt(act_c)
        M[f"d_z{l}"] = nxt(dve_c)
        M[f"d_S{l}"] = nxt(dve_c)
        if not last:
            M[f"d_xT{l}"] = nxt(dve_c)
        M[f"d_xn{l}"] = nxt(dve_c)

    with nc.Block() as block:

        @block.sync
        def _(sync):
            sync.dma_start(out=w34_sb[:].rearrange("f (l r) -> f l r", r=2 * F),
                           in_=w34_d.rearrange("l f r -> f l r")).then_inc(s_dma, 16)
            sync.dma_start(out=w5_sb[:].rearrange("f (l g) -> f l g", g=F),
                           in_=w5_d.rearrange("l f g -> f l g")).then_inc(s_dma, 16)
            sync.dma_start(out=ident[:], in_=id_d[:]).then_inc(s_dma, 16)
            sync.dma_start(out=xnat[0][:].rearrange("p (c f) -> p c f", f=F),
                           in_=x_d.rearrange("(c p) f -> p c f", p=128)).then_inc(s_dma, 16)
            sync.dma_start(out=xT[0][:], in_=xT_d[:]).then_inc(s_dma, 16)
            # store
            sync.wait_ge(s_dve, M[f"d_xn{RL-1}"])
            sync.dma_start(out=y_d.rearrange("(c p) f -> p c f", p=128),
                           in_=xnat[RL % 2][:].rearrange("p (c f) -> p c f", f=F)
                           ).then_inc(s_dma, 16)
            sync.wait_ge(s_dma, DMA_LOADS + 16)

        @block.tensor
        def _(tensor):
            for l in range(RL):
                last = l == RL - 1
                src_n, src_t = xnat[l % 2], xT[l % 2]
                WABl = w34_sb[:, ts(l % L, 2 * F)]
                W3l = w34_sb[:, (l % L) * 2 * F : (l % L) * 2 * F + F]
                W5l = w5_sb[:, ts(l % L, F)]

                # phase AB: naturals into ps0/ps1
                if l == 0:
                    tensor.wait_ge(s_dma, DMA_LOADS)
                else:
                    tensor.wait_ge(s_dve, M[f"d_xn{l-1}"])
                for c in range(NCH):
                    mmB = nc.tensor.matmul(psall[:, ts(c, 256)], src_t[:, ts(c, 128)],
                                           WABl, start=True, stop=True)
                mmB.then_inc(s_pe, 1)  # pe_AB

                # phase C: w1T into ps0
                tensor.wait_ge(s_act, M[f"a_w2n{l}"])
                for k in range(4):
                    mm = nc.tensor.matmul(ps0[:, ts(k, 512)], W3l,
                                          src_t[:, ts(k, 512)], start=True, stop=True)
                mm.then_inc(s_pe, 1)  # pe_C

                # phase S: accumulate S into ps1[:, 0:128]
                tensor.wait_ge(s_act, M[f"a_w2n{l}"])
                if l == 0:
                    tensor.wait_ge(s_dma, DMA_LOADS)
                for c in range(NCH):
                    mm = nc.tensor.matmul(ps1[:, 0:128], w2n[:, ts(c, 128)],
                                          src_n[:, ts(c, 128)],
                                          start=(c == 0), stop=(c == NCH - 1))
                mm.then_inc(s_pe, 1)  # pe_S

                # phase D: msgT into ps0 = S^T-mm + sum_c transpose(z_c)
                tensor.wait_ge(s_act, M[f"a_w1T{l}"])
                tensor.wait_ge(s_dve, M[f"d_S{l}"])
                for k in range(4):
                    nc.tensor.matmul(ps0[:, ts(k, 512)], S_sb[:], w1T[:, ts(k, 512)],
                                     start=True, stop=False, skip_group_check=True)
                for c in range(NCH):
                    mm = nc.tensor.matmul(ps0[:, ts(c, 128)], z[:, ts(c, 128)],
                                          ident[:], start=False, stop=(c % 4 == 3),
                                          is_transpose=True, skip_group_check=True)
                mm.then_inc(s_pe, 1)  # pe_D

                # phase E: yT into ps1 (skip on last layer)
                tensor.wait_ge(s_act, M[f"a_msgT{l}"])
                if not last:
                    for k in range(4):
                        mm = nc.tensor.matmul(ps1[:, ts(k, 512)], W5l,
                                              msgT[:, ts(k, 512)], start=True, stop=True)
                    mm.then_inc(s_pe, 1)  # pe_E

                # phase F: y_nat into ps0
                for c in range(NCH):
                    mm = nc.tensor.matmul(ps0[:, ts(c, 128)], msgT[:, ts(c, 128)],
                                          W5l, start=True, stop=True)
                mm.then_inc(s_pe, 1)  # pe_F

        @block.scalar
        def _(scalar):
            for l in range(RL):
                last = l == RL - 1
                scalar.wait_ge(s_pe, M[f"pe_AB{l}"])
                abv = psall[:].rearrange("p (c r) -> p c r", r=2 * F)
                nc.scalar.activation(w1n[:].rearrange("p (c f) -> p c f", f=F),
                                     abv[:, :, 0:F], AF.Prelu, alpha=SLOPE).then_inc(s_act, 1)
                nc.scalar.activation(w2n[:].rearrange("p (c f) -> p c f", f=F),
                                     abv[:, :, F : 2 * F], AF.Prelu, alpha=SLOPE).then_inc(s_act, 1)
                scalar.wait_ge(s_pe, M[f"pe_C{l}"])
                nc.scalar.activation(w1T[:], ps0[:], AF.Prelu, alpha=SLOPE
                                     ).then_inc(s_act, 1)
                scalar.wait_ge(s_pe, M[f"pe_D{l}"])
                nc.scalar.activation(msgT[:], ps0[:], AF.Copy).then_inc(s_act, 1)
                if not last:
                    scalar.wait_ge(s_pe, M[f"pe_E{l}"])
                    nc.scalar.activation(ps1[:], ps1[:], AF.Prelu, alpha=SLOPE
                                         ).then_inc(s_act, 1)
                scalar.wait_ge(s_pe, M[f"pe_F{l}"])
                nc.scalar.activation(ps0[:], ps0[:], AF.Prelu, alpha=SLOPE
                                     ).then_inc(s_act, 1)

        @block.vector
        def _(vector):
            for l in range(RL):
                last = l == RL - 1
                src_n, src_t = xnat[l % 2], xT[l % 2]
                dst_n, dst_t = xnat[(l + 1) % 2], xT[(l + 1) % 2]
                # z chain
                vector.wait_ge(s_act, M[f"a_w2n{l}"])
                nc.vector.scalar_tensor_tensor(prodneg[:], w1n[:], -1.0, w2n[:],
                                               op0=ALU.mult, op1=ALU.mult)
                nc.vector.drain()
                nc.vector.tensor_reduce(negdiag[:],
                                        prodneg[:].rearrange("p (c f) -> p c f", f=F),
                                        axis=mybir.AxisListType.X, op=ALU.add)
                nc.vector.drain()
                if l == 0:
                    vector.wait_ge(s_dma, DMA_LOADS)
                op = nc.vector.tensor_mul(
                    z[:].rearrange("p (c f) -> p c f", f=F),
                    src_n[:].rearrange("p (c f) -> p c f", f=F),
                    negdiag[:].to_broadcast([128, NCH, F]))
                nc.vector.drain()
                op.then_inc(s_dve, 1)  # d_z
                # S copy
                vector.wait_ge(s_pe, M[f"pe_S{l}"])
                nc.vector.tensor_copy(S_sb[:], ps1[:, 0:128]).then_inc(s_dve, 1)  # d_S
                # residual adds
                if not last:
                    vector.wait_ge(s_act, M[f"a_E{l}"])
                    nc.vector.scalar_tensor_tensor(dst_t[:], ps1[:], 1.0, src_t[:],
                                                   op0=ALU.mult, op1=ALU.add
                                                   ).then_inc(s_dve, 1)  # d_xT
                vector.wait_ge(s_act, M[f"a_F{l}"])
                nc.vector.scalar_tensor_tensor(dst_n[:], ps0[:], 1.0, src_n[:],
                                               op0=ALU.mult, op1=ALU.add
                                               ).then_inc(s_dve, 1)  # d_xn
                nc.vector.drain()

    ctx.close()
    return nc


def kernel(x, W3, b3, W4, b4, W5, b5, _trace=False):
    x = np.asarray(x, dtype=np.float32)
    W3 = np.ascontiguousarray(np.asarray(W3, dtype=np.float32))
    W4 = np.ascontiguousarray(np.asarray(W4, dtype=np.float32))
    W5 = np.ascontiguousarray(np.asarray(W5, dtype=np.float32))

    if "nc" not in _CACHE:
        _CACHE["nc"] = _build_nc()
    nc = _CACHE["nc"]

    w5_scaled = np.ascontiguousarray(W5 / (N - 1)).astype(np.float32)
    w34 = np.ascontiguousarray(np.concatenate([W3, W4], axis=2))
    ident = np.eye(F, dtype=np.float32)
    in_maps = []
    for b in range(B):
        in_maps.append(
            {
                "x": np.ascontiguousarray(x[b]),
                "xT": np.ascontiguousarray(x[b].T),
                "w34": w34,
                "w5": w5_scaled,
                "ident": ident,
            }
        )
    res = run_bass_kernel_spmd(nc, in_maps, list(range(B)), trace=_trace)
    out = np.stack([res.results[b]["y"] for b in range(B)], axis=0)
    if _trace:
        return out, res
    return out

